# revision 27
# baseline (speedup 1.0000x reference)
"""BitLinear (ternary-quantized linear) kernel for Trainium2, 8 NeuronCores.

Reference computation:
    scale = mean(|W|);  Wq = round(W / (scale + 1e-5));  y = (x @ Wq^T) * scale

Distribution (4x2 grid over 8 cores):
  - batch/sequence dim (8192 rows of x) split 4 ways  -> ri = core // 2
  - out_features dim (4096 rows of W) split 2 ways    -> ci = core % 2
  Each core computes y block [2048 s, 2048 o], emitted transposed (yT) so the
  matmul can keep Wq as the stationary operand; the host transposes back.
  scale needs the *global* mean(|W|): each core reduces a distinct 1/8 slice
  of W (the `wred` input) and a tiny [128] AllGather combines the partials.

Matmul structure (per core):
  x is loaded once and kept SBUF-resident, split along the contraction dim:
  the first K_BF=20 ko-chunks as bf16 (SWDGE casts in-flight), the last
  K_F8=12 as fp8e4 (hwdge fp32 loads + DVE casts).  The fp8 chunks run as
  DoubleRow matmuls (256-deep contraction per instruction, half the
  instruction count); Wq is exact in e4m3, so only x pays fp8 error and only
  on 12/32 of the contraction (measured rel err 1.78e-2 vs the 2e-2 gate;
  BITLIN_KBF trades speed for error, e.g. 16 -> 1.96e-2 and ~-25us).
  For each 128-wide block of output features, the quantized weights are the
  PE's stationary operand and the four 512-wide s-blocks stream through four
  PSUM banks (each weight load feeds 4 matmuls); the other 4 banks hold the
  previous block while it is rescaled and written out, so the PE never waits
  on eviction.  A few junk matmuls right after the gather keep the HAM clock
  gate warm through the quantize window.

Collective: the ncfw AllGather costs ~80-120us end to end (a ~30-60us
rendezvous barrier, ~10-30us of inter-op latency, and a ~18-42us data phase
— all software floors, run-to-run variable).  The wred reduction feeding it
overlaps the barrier; everything else (x loads, W staging) overlaps the
whole collective.  (A direct SDMA peer-write gather would be ~5us, but
remote_dma_broadcast hangs under this runtime's PJRT/axon path.)

Host side does layout only: slicing/transposes/swizzles of inputs,
transpose of the output blocks.  All FLOPs (reduction, quantization,
matmul, rescale) run on device.
"""

import os
import sys
import types

import numpy as np


def _ensure_axon_hooks_module():
    """Some images lack ``antenv.axon_hooks``; ``run_bass_kernel_spmd`` imports
    it unconditionally when tracing is requested. Install a no-op fallback so a
    BASS_TRACE=1 environment degrades to "no trace" instead of crashing."""
    try:
        import antenv.axon_hooks  # noqa: F401
        return
    except ImportError:
        pass
    try:
        import antenv
    except ImportError:
        return
    mod = types.ModuleType("antenv.axon_hooks")
    mod._hook = None

    def set_axon_ntff_profile_hook(h):
        mod._hook = h

    def get_axon_ntff_profile_hook():
        return mod._hook

    mod.set_axon_ntff_profile_hook = set_axon_ntff_profile_hook
    mod.get_axon_ntff_profile_hook = get_axon_ntff_profile_hook
    sys.modules["antenv.axon_hooks"] = mod
    antenv.axon_hooks = mod


_ensure_axon_hooks_module()

# ---- problem constants (hardcoded per contract) ----
B, SEQ, I_DIM, O_DIM = 4, 2048, 4096, 4096
S_TOT = B * SEQ            # 8192
R_CORES, C_CORES = 4, 2    # grid: batch x out_features
N_CORES = R_CORES * C_CORES
S_CORE = S_TOT // R_CORES  # 2048 sequence rows per core
O_CORE = O_DIM // C_CORES  # 2048 output features per core
P = 128
KP = I_DIM // P            # 32 contraction chunks
N_OB = O_CORE // P         # 16 output-feature blocks (stationary tiles)
SBLK = 512                 # s columns per matmul (one PSUM bank)
N_SB = S_CORE // SBLK      # 4 s-blocks, one PSUM bank each
# Contraction split: first K_BF ko-chunks run as bf16 matmuls, the remaining
# K_F8 run as fp8e4 DoubleRow matmuls (2 ko-chunks per instruction, 2x PE
# throughput).  Wq is exact in e4m3 (small integers); only x pays the fp8
# quantization error, and only on the K_F8 fraction of the contraction:
# predicted rel err ~= 0.024 * sqrt(K_F8/KP) against the 2e-2 gate.
K_BF = int(os.environ.get("BITLIN_KBF", "20"))
K_F8 = KP - K_BF           # must be even (DoubleRow pairs)
W_RED = O_DIM // N_CORES   # 512: columns of W^T reduced per core for mean|W|
N_RT = 16                  # wred tiles [128, 2, 512]
MAGIC = 1.5 * (2.0 ** 23)  # fp32 round-to-nearest-even trick constant
EPS = 1e-5
# The ncfw collective path is [auto-barrier, AllGather] on one stream; adding
# a warm-up AllGather in front only helps when the real gather's trigger is
# late, and costs ~20us of stream occupancy when it is early.  Off by default.
PREWARM = os.environ.get("BITLIN_PREWARM", "0") == "1"

_nc_cache = {}


def _build_kernel():
    import concourse.mybir as mybir
    import concourse.tile as tile
    from concourse import bacc
    from concourse.tile import add_dep_helper

    f32 = mybir.dt.float32
    bf16 = mybir.dt.bfloat16
    fp8 = mybir.dt.float8e4
    Alu = mybir.AluOpType
    Act = mybir.ActivationFunctionType
    DR = mybir.MatmulPerfMode.DoubleRow

    nc = bacc.Bacc(
        "TRN2",
        target_bir_lowering=False,
        debug=False,
        enable_asserts=False,
        num_devices=N_CORES,
    )

    xT = nc.dram_tensor("xT", [I_DIM, S_CORE], f32, kind="ExternalInput")
    # wTs is host-pre-swizzled to [128 part, ob, ko, 128 o] so one o-block
    # stages as a single DMA with 16 KiB contiguous per partition
    wTs = nc.dram_tensor("wTs", [P, N_OB, KP, P], f32, kind="ExternalInput")
    wred = nc.dram_tensor("wred", [I_DIM, W_RED], f32, kind="ExternalInput")
    yT = nc.dram_tensor("yT", [O_CORE, S_CORE], f32, kind="ExternalOutput")

    xT_r = xT.ap().rearrange("(ko p) s -> p ko s", p=P)      # [128, 32, 2048]
    wTs_ap = wTs.ap()
    wred_r = wred.ap().rearrange("(ko p) o -> p ko o", p=P)  # [128, 32, 512]
    yT_ap = yT.ap()

    groups = [list(range(N_CORES))]

    with tile.TileContext(nc) as tc:
        with (
            tc.tile_pool(name="const", bufs=1) as const_pool,
            tc.tile_pool(name="stats", bufs=1) as stats,
            tc.tile_pool(name="wrstage", bufs=4) as wrstage,
            tc.tile_pool(name="wstage", bufs=2) as wstage,
            tc.tile_pool(name="wq", bufs=2) as wq_pool,
            tc.tile_pool(name="xres", bufs=1) as xres_pool,
            tc.tile_pool(name="yout", bufs=4) as yout_pool,
            tc.tile_pool(name="psum", bufs=8, space="PSUM") as psum,
            tc.tile_pool(name="dram", bufs=1, space="DRAM") as dram_pool,
        ):
            # ---------- Phase 0: pre-pay the ncfw first-collective
            # rendezvous floor (~50-60us): registering the kernel-barrier
            # replica groups makes bacc insert a 1-byte AllGather right after
            # the gpsimd preamble, concurrent with wred + x loads.  (No
            # bir_kernel_barrier_wait call: nothing needs to WAIT on it, its
            # job is only to warm the collective stream.)
            if PREWARM:
                nc._bir_kernel_barrier_sem_replica_groups.extend(
                    set(g) for g in groups
                )

            # ---------- Phase A: per-partition partial sums of |wred| ----------
            # split across DVE (tensor_reduce) and ACT (Abs + accum_out)
            red_all = stats.tile([P, N_RT], f32)
            for t in range(N_RT):
                wt = wrstage.tile([P, 2, W_RED], f32, tag="wr", name=f"wr{t}")
                # split across two hwdge queues: one queue tops out ~200 GB/s
                dma_eng = nc.sync if t % 2 == 0 else nc.scalar
                dma_eng.dma_start(wt[:], wred_r[:, t * 2 : (t + 1) * 2, :])
                if t % 2 == 0:
                    nc.vector.tensor_reduce(
                        red_all[:, t : t + 1],
                        wt[:],
                        axis=mybir.AxisListType.XY,
                        op=Alu.add,
                        apply_absolute_value=True,
                    )
                else:
                    nc.scalar.activation(
                        wt[:], wt[:], Act.Abs, accum_out=red_all[:, t : t + 1]
                    )
            acc = stats.tile([P, 1], f32)
            nc.vector.tensor_reduce(
                acc[:], red_all[:], axis=mybir.AxisListType.X, op=Alu.add
            )

            # stage the first two o-blocks of W while the collective runs
            # (sync queue, ahead of the bounce/readback waits)
            wst_tiles = {}

            def stage_ob(ob):
                wst = wstage.tile([P, KP, P], f32, tag="ws", name=f"ws{ob}")
                nc.sync.dma_start(wst[:], wTs_ap[:, ob, :, :])
                wst_tiles[ob] = wst

            stage_ob(0)
            stage_ob(1)

            # ---------- Phase B1: bounce + AllGather trigger ----------
            cc_in = dram_pool.tile([P, 1], f32)
            cc_out = dram_pool.tile([N_CORES * P, 1], f32, addr_space="Shared")
            nc.sync.dma_start(cc_in[:], acc[:])
            gate = nc.gpsimd.collective_compute(
                "AllGather",
                Alu.bypass,
                replica_groups=groups,
                ins=[cc_in.opt()],
                outs=[cc_out.opt()],
            )

            # ---------- Phase D: resident x load ----------
            # bf16 part: SWDGE casts fp32 -> bf16 straight into x_bf (only
            # SWDGE can cast in-flight; plain gpsimd DMA is pinned to one
            # queue at ~250 GB/s, so it carries ONLY this part).  The DMAs
            # are ordered after the gather trigger so the wred reads that
            # feed the collective are not starved.
            # fp8 part: fp32 chunks ride the sync/scalar hwdge queues in
            # parallel (they drain behind the wred reads), and the DVE
            # casts them to fp8e4 while it is otherwise idle.
            x8 = xres_pool.tile([P, K_F8, S_CORE], fp8, name="x8")
            x_bf = xres_pool.tile([P, K_BF, S_CORE], bf16, name="x_bf")
            casts = []
            for c in range(K_F8 // 2):
                xs = wrstage.tile([P, 2, S_CORE], f32, tag="xs", bufs=2,
                                  name=f"xs{c}")
                dma_eng = nc.sync if c % 2 == 0 else nc.scalar
                dma_eng.dma_start(
                    xs[:], xT_r[:, K_BF + c * 2 : K_BF + (c + 1) * 2, :]
                )
                casts.append(
                    nc.vector.tensor_copy(x8[:, c * 2 : (c + 1) * 2, :], xs[:])
                )
            for c in range(K_BF // 4):
                xdma = nc.gpsimd.dma_start(
                    x_bf[:, c * 4 : (c + 1) * 4, :],
                    xT_r[:, c * 4 : (c + 1) * 4, :],
                )
                add_dep_helper(xdma.ins, gate.ins, sync=False,
                               reason="x load after gather trigger")

            # ---------- Phase B2: gather readback ----------
            # read back as [8, 128]: partition r = core r's [128] partials,
            # contiguous 512B per partition (8 descriptors — a [128, 8]
            # gather of scattered 4B elements costs ~30us of DMA drain).
            acc_g = stats.tile([N_CORES, P], f32)
            nc.sync.dma_start(
                acc_g[:], cc_out.rearrange("(r p) one -> r (p one)", p=P)
            )

            # ---------- Phase C: scale scalars, broadcast to all partitions ----
            # ones[8,128]^T @ acc_g[8,128] -> [128,128]: every partition holds
            # the 128 per-slot core-sums; X-reduce gives the full |W| sum on
            # every partition.  Same 8-then-128 summation tree as before (a
            # flat 1024-element sequential sum drifts from the reference).
            ones_b = const_pool.tile([N_CORES, P], f32)
            nc.vector.memset(ones_b[:], 1.0)
            ps_b = psum.tile([P, P], f32, tag="mm", name="ps_b")
            nc.tensor.matmul(ps_b[:], lhsT=ones_b[:], rhs=acc_g[:], start=True, stop=True)
            acc_r = stats.tile([P, 1], f32)
            nc.vector.tensor_reduce(
                acc_r[:], ps_b[:], axis=mybir.AxisListType.X, op=Alu.add
            )

            inv_numel = 1.0 / (float(I_DIM) * float(O_DIM))
            seps_t = stats.tile([P, 1], f32)   # scale + eps
            seps_ins = nc.vector.tensor_scalar(
                seps_t[:], acc_r[:], inv_numel, EPS, op0=Alu.mult, op1=Alu.add
            )
            # keep the x8 casts ahead of the scale chain on the DVE queue:
            # they are data-ready (~95us) before the gather lands (~105-125us)
            for cast in casts:
                add_dep_helper(seps_ins.ins, cast.ins, sync=False,
                               reason="x8 casts before scale chain on DVE")
            sinv_t = stats.tile([P, 1], f32)   # 1 / (scale + eps)
            nc.vector.reciprocal(sinv_t[:], seps_t[:])
            scale_t = stats.tile([P, 1], f32)  # mean(|W|)
            nc.vector.tensor_scalar_mul(scale_t[:], acc_r[:], inv_numel)

            # PE warm-up: the HAM clock gate needs ~3.4us of sustained PE
            # activity to unthrottle.  These junk matmuls (overwriting ps_b,
            # which the scale chain has already consumed) fill the PE-idle
            # quantize window right after the gather so ob0 starts warm.
            for _ in range(25):
                nc.tensor.matmul(ps_b[:], lhsT=ones_b[:], rhs=acc_g[:],
                                 start=True, stop=True)

            # ---------- Phase E: per-o-block quantize + matmul + evict -------
            def quantize_ob(ob):
                if ob + 2 < N_OB:
                    stage_ob(ob + 2)
                wst = wst_tiles.pop(ob)
                # bf16 chunk: ACT does wn = W*(1/(scale+eps)) + MAGIC, DVE
                # subtracts MAGIC and casts on the way out.  ob0's first
                # chunk is split off so the PE's first weight load is ready
                # ~3us sooner after the collective lands.
                wq_t = wq_pool.tile([P, K_BF, P], bf16, tag="wq", name=f"wq{ob}")
                splits = (4, K_BF) if ob == 0 else (K_BF,)
                lo = 0
                for hi in splits:
                    nc.scalar.activation(
                        wst[:, lo:hi, :], wst[:, lo:hi, :], Act.Copy,
                        bias=MAGIC, scale=sinv_t[:],
                    )
                    nc.vector.tensor_scalar_sub(
                        wq_t[:, lo:hi, :], wst[:, lo:hi, :], MAGIC
                    )
                    lo = hi
                # fp8 chunk: same trick, cast to e4m3 (exact small integers)
                wq_8 = wq_pool.tile([P, K_F8, P], fp8, tag="wq8", name=f"wq8_{ob}")
                nc.vector.tensor_scalar(
                    wst[:, K_BF:KP, :], wst[:, K_BF:KP, :], sinv_t[:], MAGIC,
                    op0=Alu.mult, op1=Alu.add,
                )
                nc.vector.tensor_scalar_sub(wq_8[:], wst[:, K_BF:KP, :], MAGIC)
                return wq_t, wq_8

            def evict(ob, sb, bank):
                yo = yout_pool.tile([P, SBLK], f32, tag="yo", name="yo")
                nc.vector.tensor_scalar_mul(yo[:], bank[:], scale_t[:])
                nc.scalar.dma_start(
                    yT_ap[ob * P : (ob + 1) * P, sb * SBLK : (sb + 1) * SBLK],
                    yo[:],
                )

            for ob in range(N_OB):
                wq_t, wq_8 = quantize_ob(ob)
                banks = [
                    psum.tile([P, SBLK], f32, tag="mm", name=f"mm{ob}_{sb}")
                    for sb in range(N_SB)
                ]
                if ob < N_OB - 1:
                    # weight-stationary order: each weight load feeds 4 MMs
                    for k in range(K_BF):
                        lhsT = wq_t[:, k, :]
                        for sb in range(N_SB):
                            nc.tensor.matmul(
                                banks[sb][:],
                                lhsT=lhsT,
                                rhs=x_bf[:, k, sb * SBLK : (sb + 1) * SBLK],
                                start=(k == 0),
                                stop=False,
                            )
                    for k2 in range(K_F8 // 2):
                        lhsT = wq_8[:, 2 * k2 : 2 * k2 + 2, :]
                        for sb in range(N_SB):
                            nc.tensor.matmul(
                                banks[sb][:],
                                lhsT=lhsT,
                                rhs=x8[:, 2 * k2 : 2 * k2 + 2,
                                       sb * SBLK : (sb + 1) * SBLK],
                                start=False,
                                stop=(k2 == K_F8 // 2 - 1),
                                perf_mode=DR,
                            )
                    for sb in range(N_SB):
                        evict(ob, sb, banks[sb])
                else:
                    # last o-block: run each s-bank's full k-chain separately
                    # so evictions+writes stagger instead of all landing after
                    # the final matmul (shorter kernel tail)
                    for sb in range(N_SB):
                        for k in range(K_BF):
                            nc.tensor.matmul(
                                banks[sb][:],
                                lhsT=wq_t[:, k, :],
                                rhs=x_bf[:, k, sb * SBLK : (sb + 1) * SBLK],
                                start=(k == 0),
                                stop=False,
                            )
                        for k2 in range(K_F8 // 2):
                            nc.tensor.matmul(
                                banks[sb][:],
                                lhsT=wq_8[:, 2 * k2 : 2 * k2 + 2, :],
                                rhs=x8[:, 2 * k2 : 2 * k2 + 2,
                                       sb * SBLK : (sb + 1) * SBLK],
                                start=False,
                                stop=(k2 == K_F8 // 2 - 1),
                                perf_mode=DR,
                            )
                        evict(ob, sb, banks[sb])

    nc.compile()
    return nc


def _get_nc():
    if "nc" not in _nc_cache:
        _nc_cache["nc"] = _build_kernel()
    return _nc_cache["nc"]


def _shard_inputs(x, W):
    x2 = np.ascontiguousarray(np.asarray(x, dtype=np.float32).reshape(S_TOT, I_DIM))
    W2 = np.ascontiguousarray(np.asarray(W, dtype=np.float32))

    xT_slices = [
        np.ascontiguousarray(x2[r * S_CORE : (r + 1) * S_CORE, :].T)
        for r in range(R_CORES)
    ]
    # wTs[p, ob, ko, o] = W[ci*O_CORE + ob*128 + o, ko*128 + p]: each o-block
    # is 16 KiB contiguous per partition in DRAM (fast staging DMA)
    wTs_slices = [
        np.ascontiguousarray(
            W2[c * O_CORE : (c + 1) * O_CORE, :]
            .reshape(N_OB, P, KP, P)
            .transpose(3, 0, 2, 1)
        )
        for c in range(C_CORES)
    ]
    wred_slices = [
        np.ascontiguousarray(W2[c * W_RED : (c + 1) * W_RED, :].T)
        for c in range(N_CORES)
    ]
    in_maps = []
    for core in range(N_CORES):
        ri, ci = core // C_CORES, core % C_CORES
        in_maps.append(
            {"xT": xT_slices[ri], "wTs": wTs_slices[ci], "wred": wred_slices[core]}
        )
    return in_maps


def _gather_output(results):
    y = np.empty((S_TOT, O_DIM), dtype=np.float32)
    for core in range(N_CORES):
        ri, ci = core // C_CORES, core % C_CORES
        y[ri * S_CORE : (ri + 1) * S_CORE, ci * O_CORE : (ci + 1) * O_CORE] = (
            results[core]["yT"].T
        )
    return y.reshape(B, SEQ, O_DIM)


def _run(x, W, **spmd_kwargs):
    import time

    from concourse.bass_utils import run_bass_kernel_spmd

    nc = _get_nc()
    in_maps = _shard_inputs(x, W)
    last_err = None
    for attempt in range(3):
        try:
            res = run_bass_kernel_spmd(
                nc, in_maps, core_ids=list(range(N_CORES)), **spmd_kwargs
            )
            return _gather_output(res.results), res
        except Exception as e:  # transient device wedges recover on retry
            last_err = e
            time.sleep(5.0 * (attempt + 1))
    raise last_err


def kernel(x, W):
    out, _ = _run(x, W)
    return out


# revision 29
# speedup vs baseline: 1.0074x; 1.0074x over previous
"""BitLinear (ternary-quantized linear) kernel for Trainium2, 8 NeuronCores.

Reference computation:
    scale = mean(|W|);  Wq = round(W / (scale + 1e-5));  y = (x @ Wq^T) * scale

Distribution (4x2 grid over 8 cores):
  - batch/sequence dim (8192 rows of x) split 4 ways  -> ri = core // 2
  - out_features dim (4096 rows of W) split 2 ways    -> ci = core % 2
  Each core computes y block [2048 s, 2048 o], emitted transposed (yT) so the
  matmul can keep Wq as the stationary operand; the host transposes back.
  scale needs the *global* mean(|W|): each core reduces a distinct 1/8 slice
  of W (the `wred` input) and a tiny [128] AllGather combines the partials.

Matmul structure (per core):
  x is loaded once and kept SBUF-resident, split along the contraction dim:
  the first K_BF=20 ko-chunks as bf16 (SWDGE casts in-flight), the last
  K_F8=12 as fp8e4 (hwdge fp32 loads + DVE casts).  The fp8 chunks run as
  DoubleRow matmuls (256-deep contraction per instruction, half the
  instruction count); Wq is exact in e4m3, so only x pays fp8 error and only
  on 12/32 of the contraction (measured rel err 1.78e-2 vs the 2e-2 gate;
  BITLIN_KBF trades speed for error, e.g. 16 -> 1.96e-2 and ~-25us).
  For each 128-wide block of output features, the quantized weights are the
  PE's stationary operand and the four 512-wide s-blocks stream through four
  PSUM banks (each weight load feeds 4 matmuls); the other 4 banks hold the
  previous block while it is rescaled and written out, so the PE never waits
  on eviction.  A few junk matmuls right after the gather keep the HAM clock
  gate warm through the quantize window.

Collective: the ncfw AllGather costs ~80-120us end to end (a ~30-60us
rendezvous barrier, ~10-30us of inter-op latency, and a ~18-42us data phase
— all software floors, run-to-run variable).  The wred reduction feeding it
overlaps the barrier; everything else (x loads, W staging) overlaps the
whole collective.  (A direct SDMA peer-write gather would be ~5us, but
remote_dma_broadcast hangs under this runtime's PJRT/axon path.)

Host side does layout only: slicing/transposes/swizzles of inputs,
transpose of the output blocks.  All FLOPs (reduction, quantization,
matmul, rescale) run on device.
"""

import os
import sys
import types

import numpy as np


def _ensure_axon_hooks_module():
    """Some images lack ``antenv.axon_hooks``; ``run_bass_kernel_spmd`` imports
    it unconditionally when tracing is requested. Install a no-op fallback so a
    BASS_TRACE=1 environment degrades to "no trace" instead of crashing."""
    try:
        import antenv.axon_hooks  # noqa: F401
        return
    except ImportError:
        pass
    try:
        import antenv
    except ImportError:
        return
    mod = types.ModuleType("antenv.axon_hooks")
    mod._hook = None

    def set_axon_ntff_profile_hook(h):
        mod._hook = h

    def get_axon_ntff_profile_hook():
        return mod._hook

    mod.set_axon_ntff_profile_hook = set_axon_ntff_profile_hook
    mod.get_axon_ntff_profile_hook = get_axon_ntff_profile_hook
    sys.modules["antenv.axon_hooks"] = mod
    antenv.axon_hooks = mod


_ensure_axon_hooks_module()

# ---- problem constants (hardcoded per contract) ----
B, SEQ, I_DIM, O_DIM = 4, 2048, 4096, 4096
S_TOT = B * SEQ            # 8192
R_CORES, C_CORES = 4, 2    # grid: batch x out_features
N_CORES = R_CORES * C_CORES
S_CORE = S_TOT // R_CORES  # 2048 sequence rows per core
O_CORE = O_DIM // C_CORES  # 2048 output features per core
P = 128
KP = I_DIM // P            # 32 contraction chunks
N_OB = O_CORE // P         # 16 output-feature blocks (stationary tiles)
SBLK = 512                 # s columns per matmul (one PSUM bank)
N_SB = S_CORE // SBLK      # 4 s-blocks, one PSUM bank each
# Contraction split: first K_BF ko-chunks run as bf16 matmuls, the remaining
# K_F8 run as fp8e4 DoubleRow matmuls (2 ko-chunks per instruction, 2x PE
# throughput).  Wq is exact in e4m3 (small integers); only x pays the fp8
# quantization error, and only on the K_F8 fraction of the contraction:
# predicted rel err ~= 0.024 * sqrt(K_F8/KP) against the 2e-2 gate.
K_BF = int(os.environ.get("BITLIN_KBF", "20"))
K_F8 = KP - K_BF           # must be even (DoubleRow pairs)
W_RED = O_DIM // N_CORES   # 512: columns of W^T reduced per core for mean|W|
N_RT = 16                  # wred tiles [128, 2, 512]
MAGIC = 1.5 * (2.0 ** 23)  # fp32 round-to-nearest-even trick constant
EPS = 1e-5
# The ncfw collective path is [auto-barrier, AllGather] on one stream; adding
# a warm-up AllGather in front only helps when the real gather's trigger is
# late, and costs ~20us of stream occupancy when it is early.  Off by default.
PREWARM = os.environ.get("BITLIN_PREWARM", "0") == "1"

_nc_cache = {}


def _build_kernel():
    import concourse.mybir as mybir
    import concourse.tile as tile
    from concourse import bacc
    from concourse.tile import add_dep_helper

    f32 = mybir.dt.float32
    bf16 = mybir.dt.bfloat16
    fp8 = mybir.dt.float8e4
    Alu = mybir.AluOpType
    Act = mybir.ActivationFunctionType
    DR = mybir.MatmulPerfMode.DoubleRow

    nc = bacc.Bacc(
        "TRN2",
        target_bir_lowering=False,
        debug=False,
        enable_asserts=False,
        num_devices=N_CORES,
    )

    xT = nc.dram_tensor("xT", [I_DIM, S_CORE], f32, kind="ExternalInput")
    # wTs is host-pre-swizzled to [128 part, ob, ko, 128 o] so one o-block
    # stages as a single DMA with 16 KiB contiguous per partition
    wTs = nc.dram_tensor("wTs", [P, N_OB, KP, P], f32, kind="ExternalInput")
    wred = nc.dram_tensor("wred", [I_DIM, W_RED], f32, kind="ExternalInput")
    yT = nc.dram_tensor("yT", [O_CORE, S_CORE], f32, kind="ExternalOutput")

    xT_r = xT.ap().rearrange("(ko p) s -> p ko s", p=P)      # [128, 32, 2048]
    wTs_ap = wTs.ap()
    wred_r = wred.ap().rearrange("(ko p) o -> p ko o", p=P)  # [128, 32, 512]
    yT_ap = yT.ap()

    groups = [list(range(N_CORES))]

    with tile.TileContext(nc) as tc:
        with (
            tc.tile_pool(name="const", bufs=1) as const_pool,
            tc.tile_pool(name="stats", bufs=1) as stats,
            tc.tile_pool(name="wrstage", bufs=4) as wrstage,
            tc.tile_pool(name="wstage", bufs=2) as wstage,
            tc.tile_pool(name="wq", bufs=2) as wq_pool,
            tc.tile_pool(name="xres", bufs=1) as xres_pool,
            tc.tile_pool(name="yout", bufs=4) as yout_pool,
            tc.tile_pool(name="psum", bufs=8, space="PSUM") as psum,
            tc.tile_pool(name="dram", bufs=1, space="DRAM") as dram_pool,
        ):
            # ---------- Phase 0: pre-pay the ncfw first-collective
            # rendezvous floor (~50-60us): registering the kernel-barrier
            # replica groups makes bacc insert a 1-byte AllGather right after
            # the gpsimd preamble, concurrent with wred + x loads.  (No
            # bir_kernel_barrier_wait call: nothing needs to WAIT on it, its
            # job is only to warm the collective stream.)
            if PREWARM:
                nc._bir_kernel_barrier_sem_replica_groups.extend(
                    set(g) for g in groups
                )

            # ---------- Phase A: per-partition partial sums of |wred| ----------
            # split across DVE (tensor_reduce) and ACT (Abs + accum_out)
            red_all = stats.tile([P, N_RT], f32)
            for t in range(N_RT):
                wt = wrstage.tile([P, 2, W_RED], f32, tag="wr", name=f"wr{t}")
                # split across two hwdge queues: one queue tops out ~200 GB/s
                dma_eng = nc.sync if t % 2 == 0 else nc.scalar
                dma_eng.dma_start(wt[:], wred_r[:, t * 2 : (t + 1) * 2, :])
                if t % 2 == 0:
                    nc.vector.tensor_reduce(
                        red_all[:, t : t + 1],
                        wt[:],
                        axis=mybir.AxisListType.XY,
                        op=Alu.add,
                        apply_absolute_value=True,
                    )
                else:
                    nc.scalar.activation(
                        wt[:], wt[:], Act.Abs, accum_out=red_all[:, t : t + 1]
                    )
            acc = stats.tile([P, 1], f32)
            acc_red = nc.vector.tensor_reduce(
                acc[:], red_all[:], axis=mybir.AxisListType.X, op=Alu.add
            )

            # stage the first two o-blocks of W while the collective runs
            # (sync queue, ahead of the bounce/readback waits)
            wst_tiles = {}

            def stage_ob(ob):
                wst = wstage.tile([P, KP, P], f32, tag="ws", name=f"ws{ob}")
                nc.sync.dma_start(wst[:], wTs_ap[:, ob, :, :])
                wst_tiles[ob] = wst

            stage_ob(0)
            stage_ob(1)

            # ---------- Phase B1: bounce + AllGather trigger ----------
            cc_in = dram_pool.tile([P, 1], f32)
            cc_out = dram_pool.tile([N_CORES * P, 1], f32, addr_space="Shared")
            nc.sync.dma_start(cc_in[:], acc[:])
            gate = nc.gpsimd.collective_compute(
                "AllGather",
                Alu.bypass,
                replica_groups=groups,
                ins=[cc_in.opt()],
                outs=[cc_out.opt()],
            )

            # ---------- Phase D: resident x load ----------
            # bf16 part: SWDGE casts fp32 -> bf16 straight into x_bf (only
            # SWDGE can cast in-flight; plain gpsimd DMA is pinned to one
            # queue at ~250 GB/s, so it carries ONLY this part).  The DMAs
            # are ordered after the gather trigger so the wred reads that
            # feed the collective are not starved.
            # fp8 part: fp32 chunks ride the sync/scalar hwdge queues in
            # parallel (they drain behind the wred reads), and the DVE
            # casts them to fp8e4 while it is otherwise idle.
            x8 = xres_pool.tile([P, K_F8, S_CORE], fp8, name="x8")
            x_bf = xres_pool.tile([P, K_BF, S_CORE], bf16, name="x_bf")
            casts = []
            for c in range(K_F8 // 2):
                xs = wrstage.tile([P, 2, S_CORE], f32, tag="xs", bufs=2,
                                  name=f"xs{c}")
                dma_eng = nc.sync if c % 2 == 0 else nc.scalar
                dma_eng.dma_start(
                    xs[:], xT_r[:, K_BF + c * 2 : K_BF + (c + 1) * 2, :]
                )
                cast = nc.vector.tensor_copy(x8[:, c * 2 : (c + 1) * 2, :], xs[:])
                # keep the casts BEHIND the |wred| reduce on the DVE queue:
                # scheduled ahead, they delay the bounce that triggers the
                # collective until their x data arrives (~40us late gather)
                add_dep_helper(cast.ins, acc_red.ins, sync=False,
                               reason="x8 casts after wred acc reduce")
                casts.append(cast)
            for c in range(K_BF // 4):
                xdma = nc.gpsimd.dma_start(
                    x_bf[:, c * 4 : (c + 1) * 4, :],
                    xT_r[:, c * 4 : (c + 1) * 4, :],
                )
                add_dep_helper(xdma.ins, gate.ins, sync=False,
                               reason="x load after gather trigger")

            # ---------- Phase B2: gather readback ----------
            # read back as [8, 128]: partition r = core r's [128] partials,
            # contiguous 512B per partition (8 descriptors — a [128, 8]
            # gather of scattered 4B elements costs ~30us of DMA drain).
            acc_g = stats.tile([N_CORES, P], f32)
            nc.sync.dma_start(
                acc_g[:], cc_out.rearrange("(r p) one -> r (p one)", p=P)
            )

            # ---------- Phase C: scale scalars, broadcast to all partitions ----
            # ones[8,128]^T @ acc_g[8,128] -> [128,128]: every partition holds
            # the 128 per-slot core-sums; X-reduce gives the full |W| sum on
            # every partition.  Same 8-then-128 summation tree as before (a
            # flat 1024-element sequential sum drifts from the reference).
            ones_b = const_pool.tile([N_CORES, P], f32)
            nc.vector.memset(ones_b[:], 1.0)
            ps_b = psum.tile([P, P], f32, tag="mm", name="ps_b")
            nc.tensor.matmul(ps_b[:], lhsT=ones_b[:], rhs=acc_g[:], start=True, stop=True)
            acc_r = stats.tile([P, 1], f32)
            nc.vector.tensor_reduce(
                acc_r[:], ps_b[:], axis=mybir.AxisListType.X, op=Alu.add
            )

            inv_numel = 1.0 / (float(I_DIM) * float(O_DIM))
            seps_t = stats.tile([P, 1], f32)   # scale + eps
            seps_ins = nc.vector.tensor_scalar(
                seps_t[:], acc_r[:], inv_numel, EPS, op0=Alu.mult, op1=Alu.add
            )
            # keep the x8 casts ahead of the scale chain on the DVE queue:
            # they are data-ready (~95us) before the gather lands (~105-125us)
            for cast in casts:
                add_dep_helper(seps_ins.ins, cast.ins, sync=False,
                               reason="x8 casts before scale chain on DVE")
            sinv_t = stats.tile([P, 1], f32)   # 1 / (scale + eps)
            nc.vector.reciprocal(sinv_t[:], seps_t[:])
            scale_t = stats.tile([P, 1], f32)  # mean(|W|)
            nc.vector.tensor_scalar_mul(scale_t[:], acc_r[:], inv_numel)

            # PE warm-up: the HAM clock gate needs ~3.4us of sustained PE
            # activity to unthrottle.  These junk matmuls (overwriting ps_b,
            # which the scale chain has already consumed) fill the PE-idle
            # quantize window right after the gather so ob0 starts warm.
            for _ in range(25):
                nc.tensor.matmul(ps_b[:], lhsT=ones_b[:], rhs=acc_g[:],
                                 start=True, stop=True)

            # ---------- Phase E: per-o-block quantize + matmul + evict -------
            def quantize_ob(ob):
                if ob + 2 < N_OB:
                    stage_ob(ob + 2)
                wst = wst_tiles.pop(ob)
                # bf16 chunk: ACT does wn = W*(1/(scale+eps)) + MAGIC, DVE
                # subtracts MAGIC and casts on the way out.  ob0's first
                # chunk is split off so the PE's first weight load is ready
                # ~3us sooner after the collective lands.
                wq_t = wq_pool.tile([P, K_BF, P], bf16, tag="wq", name=f"wq{ob}")
                splits = (4, K_BF) if ob == 0 else (K_BF,)
                lo = 0
                for hi in splits:
                    nc.scalar.activation(
                        wst[:, lo:hi, :], wst[:, lo:hi, :], Act.Copy,
                        bias=MAGIC, scale=sinv_t[:],
                    )
                    nc.vector.tensor_scalar_sub(
                        wq_t[:, lo:hi, :], wst[:, lo:hi, :], MAGIC
                    )
                    lo = hi
                # fp8 chunk: same trick, cast to e4m3 (exact small integers)
                wq_8 = wq_pool.tile([P, K_F8, P], fp8, tag="wq8", name=f"wq8_{ob}")
                nc.vector.tensor_scalar(
                    wst[:, K_BF:KP, :], wst[:, K_BF:KP, :], sinv_t[:], MAGIC,
                    op0=Alu.mult, op1=Alu.add,
                )
                nc.vector.tensor_scalar_sub(wq_8[:], wst[:, K_BF:KP, :], MAGIC)
                return wq_t, wq_8

            def evict(ob, sb, bank):
                yo = yout_pool.tile([P, SBLK], f32, tag="yo", name="yo")
                nc.vector.tensor_scalar_mul(yo[:], bank[:], scale_t[:])
                nc.scalar.dma_start(
                    yT_ap[ob * P : (ob + 1) * P, sb * SBLK : (sb + 1) * SBLK],
                    yo[:],
                )

            for ob in range(N_OB):
                wq_t, wq_8 = quantize_ob(ob)
                banks = [
                    psum.tile([P, SBLK], f32, tag="mm", name=f"mm{ob}_{sb}")
                    for sb in range(N_SB)
                ]
                if ob < N_OB - 1:
                    # weight-stationary order: each weight load feeds 4 MMs
                    for k in range(K_BF):
                        lhsT = wq_t[:, k, :]
                        for sb in range(N_SB):
                            nc.tensor.matmul(
                                banks[sb][:],
                                lhsT=lhsT,
                                rhs=x_bf[:, k, sb * SBLK : (sb + 1) * SBLK],
                                start=(k == 0),
                                stop=False,
                            )
                    for k2 in range(K_F8 // 2):
                        lhsT = wq_8[:, 2 * k2 : 2 * k2 + 2, :]
                        for sb in range(N_SB):
                            nc.tensor.matmul(
                                banks[sb][:],
                                lhsT=lhsT,
                                rhs=x8[:, 2 * k2 : 2 * k2 + 2,
                                       sb * SBLK : (sb + 1) * SBLK],
                                start=False,
                                stop=(k2 == K_F8 // 2 - 1),
                                perf_mode=DR,
                            )
                    for sb in range(N_SB):
                        evict(ob, sb, banks[sb])
                else:
                    # last o-block: run each s-bank's full k-chain separately
                    # so evictions+writes stagger instead of all landing after
                    # the final matmul (shorter kernel tail)
                    for sb in range(N_SB):
                        for k in range(K_BF):
                            nc.tensor.matmul(
                                banks[sb][:],
                                lhsT=wq_t[:, k, :],
                                rhs=x_bf[:, k, sb * SBLK : (sb + 1) * SBLK],
                                start=(k == 0),
                                stop=False,
                            )
                        for k2 in range(K_F8 // 2):
                            nc.tensor.matmul(
                                banks[sb][:],
                                lhsT=wq_8[:, 2 * k2 : 2 * k2 + 2, :],
                                rhs=x8[:, 2 * k2 : 2 * k2 + 2,
                                       sb * SBLK : (sb + 1) * SBLK],
                                start=False,
                                stop=(k2 == K_F8 // 2 - 1),
                                perf_mode=DR,
                            )
                        evict(ob, sb, banks[sb])

    nc.compile()
    return nc


def _get_nc():
    if "nc" not in _nc_cache:
        _nc_cache["nc"] = _build_kernel()
    return _nc_cache["nc"]


def _shard_inputs(x, W):
    x2 = np.ascontiguousarray(np.asarray(x, dtype=np.float32).reshape(S_TOT, I_DIM))
    W2 = np.ascontiguousarray(np.asarray(W, dtype=np.float32))

    xT_slices = [
        np.ascontiguousarray(x2[r * S_CORE : (r + 1) * S_CORE, :].T)
        for r in range(R_CORES)
    ]
    # wTs[p, ob, ko, o] = W[ci*O_CORE + ob*128 + o, ko*128 + p]: each o-block
    # is 16 KiB contiguous per partition in DRAM (fast staging DMA)
    wTs_slices = [
        np.ascontiguousarray(
            W2[c * O_CORE : (c + 1) * O_CORE, :]
            .reshape(N_OB, P, KP, P)
            .transpose(3, 0, 2, 1)
        )
        for c in range(C_CORES)
    ]
    wred_slices = [
        np.ascontiguousarray(W2[c * W_RED : (c + 1) * W_RED, :].T)
        for c in range(N_CORES)
    ]
    in_maps = []
    for core in range(N_CORES):
        ri, ci = core // C_CORES, core % C_CORES
        in_maps.append(
            {"xT": xT_slices[ri], "wTs": wTs_slices[ci], "wred": wred_slices[core]}
        )
    return in_maps


def _gather_output(results):
    y = np.empty((S_TOT, O_DIM), dtype=np.float32)
    for core in range(N_CORES):
        ri, ci = core // C_CORES, core % C_CORES
        y[ri * S_CORE : (ri + 1) * S_CORE, ci * O_CORE : (ci + 1) * O_CORE] = (
            results[core]["yT"].T
        )
    return y.reshape(B, SEQ, O_DIM)


def _run(x, W, **spmd_kwargs):
    import time

    from concourse.bass_utils import run_bass_kernel_spmd

    nc = _get_nc()
    in_maps = _shard_inputs(x, W)
    last_err = None
    for attempt in range(3):
        try:
            res = run_bass_kernel_spmd(
                nc, in_maps, core_ids=list(range(N_CORES)), **spmd_kwargs
            )
            return _gather_output(res.results), res
        except Exception as e:  # transient device wedges recover on retry
            last_err = e
            time.sleep(5.0 * (attempt + 1))
    raise last_err


def kernel(x, W):
    out, _ = _run(x, W)
    return out


# revision 31
# speedup vs baseline: 1.0129x; 1.0055x over previous
"""BitLinear (ternary-quantized linear) kernel for Trainium2, 8 NeuronCores.

Reference computation:
    scale = mean(|W|);  Wq = round(W / (scale + 1e-5));  y = (x @ Wq^T) * scale

Distribution (4x2 grid over 8 cores):
  - batch/sequence dim (8192 rows of x) split 4 ways  -> ri = core // 2
  - out_features dim (4096 rows of W) split 2 ways    -> ci = core % 2
  Each core computes y block [2048 s, 2048 o], emitted transposed (yT) so the
  matmul can keep Wq as the stationary operand; the host transposes back.
  scale needs the *global* mean(|W|): each core reduces a distinct 1/8 slice
  of W (the `wred` input) and a tiny [128] AllGather combines the partials.

Matmul structure (per core):
  x is loaded once and kept SBUF-resident, split along the contraction dim:
  the first K_BF=20 ko-chunks as bf16 (SWDGE casts in-flight), the last
  K_F8=12 as fp8e4 (hwdge fp32 loads + DVE casts).  The fp8 chunks run as
  DoubleRow matmuls (256-deep contraction per instruction, half the
  instruction count); Wq is exact in e4m3, so only x pays fp8 error and only
  on 12/32 of the contraction (measured rel err 1.78e-2 vs the 2e-2 gate;
  BITLIN_KBF trades speed for error, e.g. 16 -> 1.96e-2 and ~-25us).
  For each 128-wide block of output features, the quantized weights are the
  PE's stationary operand and the four 512-wide s-blocks stream through four
  PSUM banks (each weight load feeds 4 matmuls); the other 4 banks hold the
  previous block while it is rescaled and written out, so the PE never waits
  on eviction.  A few junk matmuls right after the gather keep the HAM clock
  gate warm through the quantize window.

Collective: the ncfw AllGather costs ~80-120us end to end (a ~30-60us
rendezvous barrier, ~10-30us of inter-op latency, and a ~18-42us data phase
— all software floors, run-to-run variable).  The wred reduction feeding it
overlaps the barrier; everything else (x loads, W staging) overlaps the
whole collective.  (A direct SDMA peer-write gather would be ~5us, but
remote_dma_broadcast hangs under this runtime's PJRT/axon path.)

Host side does layout only: slicing/transposes/swizzles of inputs,
transpose of the output blocks.  All FLOPs (reduction, quantization,
matmul, rescale) run on device.
"""

import os
import sys
import types

import numpy as np


def _ensure_axon_hooks_module():
    """Some images lack ``antenv.axon_hooks``; ``run_bass_kernel_spmd`` imports
    it unconditionally when tracing is requested. Install a no-op fallback so a
    BASS_TRACE=1 environment degrades to "no trace" instead of crashing."""
    try:
        import antenv.axon_hooks  # noqa: F401
        return
    except ImportError:
        pass
    try:
        import antenv
    except ImportError:
        return
    mod = types.ModuleType("antenv.axon_hooks")
    mod._hook = None

    def set_axon_ntff_profile_hook(h):
        mod._hook = h

    def get_axon_ntff_profile_hook():
        return mod._hook

    mod.set_axon_ntff_profile_hook = set_axon_ntff_profile_hook
    mod.get_axon_ntff_profile_hook = get_axon_ntff_profile_hook
    sys.modules["antenv.axon_hooks"] = mod
    antenv.axon_hooks = mod


_ensure_axon_hooks_module()

# ---- problem constants (hardcoded per contract) ----
B, SEQ, I_DIM, O_DIM = 4, 2048, 4096, 4096
S_TOT = B * SEQ            # 8192
R_CORES, C_CORES = 4, 2    # grid: batch x out_features
N_CORES = R_CORES * C_CORES
S_CORE = S_TOT // R_CORES  # 2048 sequence rows per core
O_CORE = O_DIM // C_CORES  # 2048 output features per core
P = 128
KP = I_DIM // P            # 32 contraction chunks
N_OB = O_CORE // P         # 16 output-feature blocks (stationary tiles)
SBLK = 512                 # s columns per matmul (one PSUM bank)
N_SB = S_CORE // SBLK      # 4 s-blocks, one PSUM bank each
# Contraction split: first K_BF ko-chunks run as bf16 matmuls, the remaining
# K_F8 run as fp8e4 DoubleRow matmuls (2 ko-chunks per instruction, 2x PE
# throughput).  Wq is exact in e4m3 (small integers); only x pays the fp8
# quantization error, and only on the K_F8 fraction of the contraction:
# predicted rel err ~= 0.024 * sqrt(K_F8/KP) against the 2e-2 gate.
K_BF = int(os.environ.get("BITLIN_KBF", "20"))
K_F8 = KP - K_BF           # must be even (DoubleRow pairs)
W_RED = O_DIM // N_CORES   # 512: columns of W^T reduced per core for mean|W|
N_RT = 16                  # wred tiles [128, 2, 512]
MAGIC = 1.5 * (2.0 ** 23)  # fp32 round-to-nearest-even trick constant
EPS = 1e-5
# The ncfw collective path is [auto-barrier, AllGather] on one stream; adding
# a warm-up AllGather in front only helps when the real gather's trigger is
# late, and costs ~20us of stream occupancy when it is early.  Off by default.
PREWARM = os.environ.get("BITLIN_PREWARM", "0") == "1"

_nc_cache = {}


def _build_kernel():
    import concourse.mybir as mybir
    import concourse.tile as tile
    from concourse import bacc
    from concourse.tile import add_dep_helper

    f32 = mybir.dt.float32
    bf16 = mybir.dt.bfloat16
    fp8 = mybir.dt.float8e4
    Alu = mybir.AluOpType
    Act = mybir.ActivationFunctionType
    DR = mybir.MatmulPerfMode.DoubleRow

    nc = bacc.Bacc(
        "TRN2",
        target_bir_lowering=False,
        debug=False,
        enable_asserts=False,
        num_devices=N_CORES,
    )

    xT = nc.dram_tensor("xT", [I_DIM, S_CORE], f32, kind="ExternalInput")
    # wTs is host-pre-swizzled to [128 part, ob, ko, 128 o] so one o-block
    # stages as a single DMA with 16 KiB contiguous per partition
    wTs = nc.dram_tensor("wTs", [P, N_OB, KP, P], f32, kind="ExternalInput")
    wred = nc.dram_tensor("wred", [I_DIM, W_RED], f32, kind="ExternalInput")
    yT = nc.dram_tensor("yT", [O_CORE, S_CORE], f32, kind="ExternalOutput")

    xT_r = xT.ap().rearrange("(ko p) s -> p ko s", p=P)      # [128, 32, 2048]
    wTs_ap = wTs.ap()
    wred_r = wred.ap().rearrange("(ko p) o -> p ko o", p=P)  # [128, 32, 512]
    yT_ap = yT.ap()

    groups = [list(range(N_CORES))]

    with tile.TileContext(nc) as tc:
        with (
            tc.tile_pool(name="const", bufs=1) as const_pool,
            tc.tile_pool(name="stats", bufs=1) as stats,
            tc.tile_pool(name="wrstage", bufs=4) as wrstage,
            tc.tile_pool(name="wstage", bufs=2) as wstage,
            tc.tile_pool(name="wq", bufs=2) as wq_pool,
            tc.tile_pool(name="xres", bufs=1) as xres_pool,
            tc.tile_pool(name="yout", bufs=4) as yout_pool,
            tc.tile_pool(name="psum", bufs=8, space="PSUM") as psum,
            tc.tile_pool(name="dram", bufs=1, space="DRAM") as dram_pool,
        ):
            # ---------- Phase 0: pre-pay the ncfw first-collective
            # rendezvous floor (~50-60us): registering the kernel-barrier
            # replica groups makes bacc insert a 1-byte AllGather right after
            # the gpsimd preamble, concurrent with wred + x loads.  (No
            # bir_kernel_barrier_wait call: nothing needs to WAIT on it, its
            # job is only to warm the collective stream.)
            if PREWARM:
                nc._bir_kernel_barrier_sem_replica_groups.extend(
                    set(g) for g in groups
                )

            # ---------- Phase A: per-partition partial sums of |wred| ----------
            # split across DVE (tensor_reduce) and ACT (Abs + accum_out)
            red_all = stats.tile([P, N_RT], f32)
            for t in range(N_RT):
                wt = wrstage.tile([P, 2, W_RED], f32, tag="wr", name=f"wr{t}")
                # split across two hwdge queues: one queue tops out ~200 GB/s
                dma_eng = nc.sync if t % 2 == 0 else nc.scalar
                dma_eng.dma_start(wt[:], wred_r[:, t * 2 : (t + 1) * 2, :])
                if t % 2 == 0:
                    nc.vector.tensor_reduce(
                        red_all[:, t : t + 1],
                        wt[:],
                        axis=mybir.AxisListType.XY,
                        op=Alu.add,
                        apply_absolute_value=True,
                    )
                else:
                    nc.scalar.activation(
                        wt[:], wt[:], Act.Abs, accum_out=red_all[:, t : t + 1]
                    )
            acc = stats.tile([P, 1], f32)
            acc_red = nc.vector.tensor_reduce(
                acc[:], red_all[:], axis=mybir.AxisListType.X, op=Alu.add
            )

            wst_tiles = {}

            def stage_ob(ob):
                wst = wstage.tile([P, KP, P], f32, tag="ws", name=f"ws{ob}")
                nc.sync.dma_start(wst[:], wTs_ap[:, ob, :, :])
                wst_tiles[ob] = wst

            # ---------- Phase B1: bounce + AllGather trigger ----------
            cc_in = dram_pool.tile([P, 1], f32)
            cc_out = dram_pool.tile([N_CORES * P, 1], f32, addr_space="Shared")
            nc.sync.dma_start(cc_in[:], acc[:])
            gate = nc.gpsimd.collective_compute(
                "AllGather",
                Alu.bypass,
                replica_groups=groups,
                ins=[cc_in.opt()],
                outs=[cc_out.opt()],
            )

            # stage the first two o-blocks of W while the collective runs
            # (after the bounce on the sync queue: 4 MB of W ahead of the
            # bounce's 512B transfer delays the gather trigger ~20us)
            stage_ob(0)
            stage_ob(1)

            # ---------- Phase D: resident x load ----------
            # bf16 part: SWDGE casts fp32 -> bf16 straight into x_bf (only
            # SWDGE can cast in-flight; plain gpsimd DMA is pinned to one
            # queue at ~250 GB/s, so it carries ONLY this part).  The DMAs
            # are ordered after the gather trigger so the wred reads that
            # feed the collective are not starved.
            # fp8 part: fp32 chunks ride the sync/scalar hwdge queues in
            # parallel (they drain behind the wred reads), and the DVE
            # casts them to fp8e4 while it is otherwise idle.
            x8 = xres_pool.tile([P, K_F8, S_CORE], fp8, name="x8")
            x_bf = xres_pool.tile([P, K_BF, S_CORE], bf16, name="x_bf")
            casts = []
            for c in range(K_F8 // 2):
                xs = wrstage.tile([P, 2, S_CORE], f32, tag="xs", bufs=2,
                                  name=f"xs{c}")
                dma_eng = nc.sync if c % 2 == 0 else nc.scalar
                dma_eng.dma_start(
                    xs[:], xT_r[:, K_BF + c * 2 : K_BF + (c + 1) * 2, :]
                )
                cast = nc.vector.tensor_copy(x8[:, c * 2 : (c + 1) * 2, :], xs[:])
                # keep the casts BEHIND the |wred| reduce on the DVE queue:
                # scheduled ahead, they delay the bounce that triggers the
                # collective until their x data arrives (~40us late gather)
                add_dep_helper(cast.ins, acc_red.ins, sync=False,
                               reason="x8 casts after wred acc reduce")
                casts.append(cast)
            for c in range(K_BF // 4):
                xdma = nc.gpsimd.dma_start(
                    x_bf[:, c * 4 : (c + 1) * 4, :],
                    xT_r[:, c * 4 : (c + 1) * 4, :],
                )
                add_dep_helper(xdma.ins, gate.ins, sync=False,
                               reason="x load after gather trigger")

            # ---------- Phase B2: gather readback ----------
            # read back as [8, 128]: partition r = core r's [128] partials,
            # contiguous 512B per partition (8 descriptors — a [128, 8]
            # gather of scattered 4B elements costs ~30us of DMA drain).
            acc_g = stats.tile([N_CORES, P], f32)
            nc.sync.dma_start(
                acc_g[:], cc_out.rearrange("(r p) one -> r (p one)", p=P)
            )

            # ---------- Phase C: scale scalars, broadcast to all partitions ----
            # ones[8,128]^T @ acc_g[8,128] -> [128,128]: every partition holds
            # the 128 per-slot core-sums; X-reduce gives the full |W| sum on
            # every partition.  Same 8-then-128 summation tree as before (a
            # flat 1024-element sequential sum drifts from the reference).
            ones_b = const_pool.tile([N_CORES, P], f32)
            nc.vector.memset(ones_b[:], 1.0)
            ps_b = psum.tile([P, P], f32, tag="mm", name="ps_b")
            nc.tensor.matmul(ps_b[:], lhsT=ones_b[:], rhs=acc_g[:], start=True, stop=True)
            acc_r = stats.tile([P, 1], f32)
            nc.vector.tensor_reduce(
                acc_r[:], ps_b[:], axis=mybir.AxisListType.X, op=Alu.add
            )

            inv_numel = 1.0 / (float(I_DIM) * float(O_DIM))
            seps_t = stats.tile([P, 1], f32)   # scale + eps
            seps_ins = nc.vector.tensor_scalar(
                seps_t[:], acc_r[:], inv_numel, EPS, op0=Alu.mult, op1=Alu.add
            )
            # keep the x8 casts ahead of the scale chain on the DVE queue:
            # they are data-ready (~95us) before the gather lands (~105-125us)
            for cast in casts:
                add_dep_helper(seps_ins.ins, cast.ins, sync=False,
                               reason="x8 casts before scale chain on DVE")
            sinv_t = stats.tile([P, 1], f32)   # 1 / (scale + eps)
            nc.vector.reciprocal(sinv_t[:], seps_t[:])
            scale_t = stats.tile([P, 1], f32)  # mean(|W|)
            nc.vector.tensor_scalar_mul(scale_t[:], acc_r[:], inv_numel)

            # PE warm-up: the HAM clock gate needs ~3.4us of sustained PE
            # activity to unthrottle.  These junk matmuls (overwriting ps_b,
            # which the scale chain has already consumed) fill the PE-idle
            # quantize window right after the gather so ob0 starts warm.
            for _ in range(25):
                nc.tensor.matmul(ps_b[:], lhsT=ones_b[:], rhs=acc_g[:],
                                 start=True, stop=True)

            # ---------- Phase E: per-o-block quantize + matmul + evict -------
            def quantize_ob(ob):
                if ob + 2 < N_OB:
                    stage_ob(ob + 2)
                wst = wst_tiles.pop(ob)
                # bf16 chunk: ACT does wn = W*(1/(scale+eps)) + MAGIC, DVE
                # subtracts MAGIC and casts on the way out.  ob0's first
                # chunk is split off so the PE's first weight load is ready
                # ~3us sooner after the collective lands.
                wq_t = wq_pool.tile([P, K_BF, P], bf16, tag="wq", name=f"wq{ob}")
                splits = (4, K_BF) if ob == 0 else (K_BF,)
                lo = 0
                for hi in splits:
                    nc.scalar.activation(
                        wst[:, lo:hi, :], wst[:, lo:hi, :], Act.Copy,
                        bias=MAGIC, scale=sinv_t[:],
                    )
                    nc.vector.tensor_scalar_sub(
                        wq_t[:, lo:hi, :], wst[:, lo:hi, :], MAGIC
                    )
                    lo = hi
                # fp8 chunk: same trick, cast to e4m3 (exact small integers)
                wq_8 = wq_pool.tile([P, K_F8, P], fp8, tag="wq8", name=f"wq8_{ob}")
                nc.vector.tensor_scalar(
                    wst[:, K_BF:KP, :], wst[:, K_BF:KP, :], sinv_t[:], MAGIC,
                    op0=Alu.mult, op1=Alu.add,
                )
                nc.vector.tensor_scalar_sub(wq_8[:], wst[:, K_BF:KP, :], MAGIC)
                return wq_t, wq_8

            def evict(ob, sb, bank):
                yo = yout_pool.tile([P, SBLK], f32, tag="yo", name="yo")
                nc.vector.tensor_scalar_mul(yo[:], bank[:], scale_t[:])
                nc.scalar.dma_start(
                    yT_ap[ob * P : (ob + 1) * P, sb * SBLK : (sb + 1) * SBLK],
                    yo[:],
                )

            for ob in range(N_OB):
                wq_t, wq_8 = quantize_ob(ob)
                banks = [
                    psum.tile([P, SBLK], f32, tag="mm", name=f"mm{ob}_{sb}")
                    for sb in range(N_SB)
                ]
                if ob < N_OB - 1:
                    # weight-stationary order: each weight load feeds 4 MMs
                    for k in range(K_BF):
                        lhsT = wq_t[:, k, :]
                        for sb in range(N_SB):
                            nc.tensor.matmul(
                                banks[sb][:],
                                lhsT=lhsT,
                                rhs=x_bf[:, k, sb * SBLK : (sb + 1) * SBLK],
                                start=(k == 0),
                                stop=False,
                            )
                    for k2 in range(K_F8 // 2):
                        lhsT = wq_8[:, 2 * k2 : 2 * k2 + 2, :]
                        for sb in range(N_SB):
                            nc.tensor.matmul(
                                banks[sb][:],
                                lhsT=lhsT,
                                rhs=x8[:, 2 * k2 : 2 * k2 + 2,
                                       sb * SBLK : (sb + 1) * SBLK],
                                start=False,
                                stop=(k2 == K_F8 // 2 - 1),
                                perf_mode=DR,
                            )
                    for sb in range(N_SB):
                        evict(ob, sb, banks[sb])
                else:
                    # last o-block: run each s-bank's full k-chain separately
                    # so evictions+writes stagger instead of all landing after
                    # the final matmul (shorter kernel tail)
                    for sb in range(N_SB):
                        for k in range(K_BF):
                            nc.tensor.matmul(
                                banks[sb][:],
                                lhsT=wq_t[:, k, :],
                                rhs=x_bf[:, k, sb * SBLK : (sb + 1) * SBLK],
                                start=(k == 0),
                                stop=False,
                            )
                        for k2 in range(K_F8 // 2):
                            nc.tensor.matmul(
                                banks[sb][:],
                                lhsT=wq_8[:, 2 * k2 : 2 * k2 + 2, :],
                                rhs=x8[:, 2 * k2 : 2 * k2 + 2,
                                       sb * SBLK : (sb + 1) * SBLK],
                                start=False,
                                stop=(k2 == K_F8 // 2 - 1),
                                perf_mode=DR,
                            )
                        evict(ob, sb, banks[sb])

    nc.compile()
    return nc


def _get_nc():
    if "nc" not in _nc_cache:
        _nc_cache["nc"] = _build_kernel()
    return _nc_cache["nc"]


def _shard_inputs(x, W):
    x2 = np.ascontiguousarray(np.asarray(x, dtype=np.float32).reshape(S_TOT, I_DIM))
    W2 = np.ascontiguousarray(np.asarray(W, dtype=np.float32))

    xT_slices = [
        np.ascontiguousarray(x2[r * S_CORE : (r + 1) * S_CORE, :].T)
        for r in range(R_CORES)
    ]
    # wTs[p, ob, ko, o] = W[ci*O_CORE + ob*128 + o, ko*128 + p]: each o-block
    # is 16 KiB contiguous per partition in DRAM (fast staging DMA)
    wTs_slices = [
        np.ascontiguousarray(
            W2[c * O_CORE : (c + 1) * O_CORE, :]
            .reshape(N_OB, P, KP, P)
            .transpose(3, 0, 2, 1)
        )
        for c in range(C_CORES)
    ]
    wred_slices = [
        np.ascontiguousarray(W2[c * W_RED : (c + 1) * W_RED, :].T)
        for c in range(N_CORES)
    ]
    in_maps = []
    for core in range(N_CORES):
        ri, ci = core // C_CORES, core % C_CORES
        in_maps.append(
            {"xT": xT_slices[ri], "wTs": wTs_slices[ci], "wred": wred_slices[core]}
        )
    return in_maps


def _gather_output(results):
    y = np.empty((S_TOT, O_DIM), dtype=np.float32)
    for core in range(N_CORES):
        ri, ci = core // C_CORES, core % C_CORES
        y[ri * S_CORE : (ri + 1) * S_CORE, ci * O_CORE : (ci + 1) * O_CORE] = (
            results[core]["yT"].T
        )
    return y.reshape(B, SEQ, O_DIM)


def _run(x, W, **spmd_kwargs):
    import time

    from concourse.bass_utils import run_bass_kernel_spmd

    nc = _get_nc()
    in_maps = _shard_inputs(x, W)
    last_err = None
    for attempt in range(3):
        try:
            res = run_bass_kernel_spmd(
                nc, in_maps, core_ids=list(range(N_CORES)), **spmd_kwargs
            )
            return _gather_output(res.results), res
        except Exception as e:  # transient device wedges recover on retry
            last_err = e
            time.sleep(5.0 * (attempt + 1))
    raise last_err


def kernel(x, W):
    out, _ = _run(x, W)
    return out
